# revision 43
# baseline (speedup 1.0000x reference)
"""Bathtub reconstructor Trainium2 kernel.

Reference does, per (b, y, x, t) cell with its 16 fine topo values z_k:
    solve mean(relu(h - z)) = d by 20-step bisection, output relu(h - z_k).

Key identity (water-filling): with z sorted ascending and P_j = z_1+...+z_j,
    sum_k relu(h - z_k) = max_j (j*h - P_j)   (convex, increasing)
so the root of sum = 16*d is exactly
    h* = min_{j=1..16} (16*d + P_j) / j = min_j (a_j * d + b_j),
with a_j = 16/j (constants) and b_j = P_j/j (per-cell constants).
This replaces the 20-iteration bisection with 16 fused multiply-adds and a
16-way min, then the output pass relu(h* - z_k).

Device layout (per core, n_y sharded 8 ways -> 8 y-rows/core):
  partitions = 128 cells (4 tiles cover the 512 (y,x) cells)
  free dim   = 512 combos (b-major: b*32 + t)
  stage1: hj[j] = a_j*d + b_j   (scalar ACT: Identity w/ scale+bias, and
                                 vector tensor_scalar mult+add, split)
  stage2: h = min over j        (vector tensor_reduce, j innermost via AP)
  stage3: out[k] = relu(h - z_k) (vector tensor_scalar add+max / ACT Relu)
All DMAs fully contiguous; host pre/post-permutes (cheap numpy).
"""

import numpy as np

import concourse.bass as bass
import concourse.tile as tile
from concourse import bacc, dve_ops, mybir
from concourse.bass_utils import run_bass_kernel_spmd
from concourse.dve_ops import OPS, DveOp, get_dve_sub_opcode, has_src1
from concourse.dve_spec import C0, C1, Spec, Src0, Src1, lower, minn
from concourse.dve_uop import DveOpSpec


def _register_op(name, spec) -> DveOp:
    for o in OPS:
        if o.name == name:
            return o
    op = DveOp(name, spec, subdim=False, uops_sha={})
    OPS.append(op)
    dve_ops.CUSTOM_DVE_SPECS[op.name] = op.spec
    dve_ops._SUB_OPCODE_FOR_NAME[op.name] = (
        dve_ops._CUSTOM_DVE_ROW_BASE + len(OPS) - 1
    )
    for ver in ("v3", "v4"):
        tmp = DveOpSpec(
            name=op.name,
            opcode=get_dve_sub_opcode(op.name),
            uops=lower(spec, ver=ver),
            rd1_en=has_src1(spec),
        )
        op.uops_sha[ver] = tmp.sha(ver)
    return op


def _register_affine_min() -> DveOp:
    """Custom fused DVE op: out = min(in0*s0 + s1, in1).

    One [128,512] 1x-rate instruction per water-level line replaces a
    tensor_scalar (affine) + tensor_tensor (min-tree level) pair: the
    16-line lower envelope becomes a min-accumulate chain.
    """
    return _register_op(
        "AFFINE_THEN_MIN",
        Spec(
            body=minn(Src0 * C0 + C1, Src1),
            reference=lambda in0, in1, s0, s1, imm2: np.minimum(
                in0.astype(np.float32) * s0 + s1, in1
            ),
        ),
    )


def _register_pair_seed() -> DveOp:
    """Custom fused DVE op: out = min(in0*s0 + s1, in0*imm2 + latch(in1)).

    Two envelope lines in one instruction: the 4th scalar (second line's
    bias) rides the otherwise-unused Src1 stream, latched at element 0,
    so the chain seed covers lines 0 and 1 together.
    """
    from concourse.dve_spec import _spill_c3_to_src1, C2, C3

    body = minn(Src0 * C0 + C1, Src0 * C2 + C3)
    return _register_op(
        "AFFINE_PAIR_MIN",
        Spec(
            body=_spill_c3_to_src1(body),
            reference=lambda in0, in1, s0, s1, imm2: np.minimum(
                in0.astype(np.float32) * s0 + s1,
                in0.astype(np.float32) * imm2 + in1,
            ),
        ),
    )

BS, NY, NX, NT, F = 16, 64, 64, 32, 4
FF = F * F                # 16 fine cells per coarse cell
NCORES = 8
YPC = NY // NCORES        # 8 coarse y rows per core
CELLS = YPC * NX          # 512 cells per core
NCT = CELLS // 128        # 4 cell-tiles of 128 partitions
COMBOS = BS * NT          # 512 (b, t) combos per cell

F32 = mybir.dt.float32

# Engine split: vector runs the fused affine+min chain (stage1+2), scalar
# runs stage3 relu acts (~707ns each). GpSimd is unusable here: its
# tensor_scalar path measured 8.3us/op and its SBUF-port contention
# starved the DVE 12x.
S3_VEC = 2    # stage3: last S3_VEC k's on vector in steady units

_CACHE = {}


def _build_nc():
    fmin = _register_affine_min()
    fpair = _register_pair_seed()
    nc = bacc.Bacc(
        "TRN2", target_bir_lowering=False, debug=False, num_devices=NCORES
    )
    u_ext = nc.declare_dram_parameter("u", [CELLS, COMBOS], F32, isOutput=False)
    # per-cell constants, interleaved: cols 0:16 = b_j = P_j/j, 16:32 = -z_k
    cf_ext = nc.declare_dram_parameter("coef", [CELLS, 2 * FF], F32, isOutput=False)
    out_ext = nc.declare_dram_parameter(
        "out", [CELLS, FF * COMBOS], F32, isOutput=True
    )

    a = [float(FF) / j for j in range(1, FF + 1)]

    with tile.TileContext(nc) as tc:
        with (
            tc.tile_pool(name="dpool", bufs=4) as dpool,
            tc.tile_pool(name="cfpool", bufs=4) as cfpool,
            tc.tile_pool(name="accpool", bufs=2) as accpool,
            tc.tile_pool(name="hpool", bufs=3) as hpool,
            tc.tile_pool(name="opool", bufs=4) as opool,
        ):
            # (tile index, combo slice): tile 0 is split in two along combos
            # so the first h (and the first output DMA) lands ~5us earlier --
            # the kernel end is paced by the 17MB HBM output stream, so
            # starting it early is worth the extra instruction overheads.
            units = [(ct, 0, COMBOS) for ct in range(NCT)]
            d_tiles = {}
            cf_tiles = {}
            for ui, (ct, c0, c1) in enumerate(units):
                rows = slice(128 * ct, 128 * (ct + 1))
                cw = c1 - c0

                if ct not in d_tiles:
                    dt = dpool.tile([128, COMBOS], F32)
                    nc.sync.dma_start(dt[:], u_ext[rows, :])
                    cft = cfpool.tile([128, 2 * FF], F32)
                    nc.sync.dma_start(cft[:], cf_ext[rows, :])
                    d_tiles[ct] = dt
                    cf_tiles[ct] = cft
                d = d_tiles[ct][:, c0:c1]
                cf = cf_tiles[ct]
                nz = cf[:, FF:2 * FF]

                # stage1+2 fused: h = min_j (a_j*d + b_j) via TWO interleaved
                # min-accumulate chains (consecutive vector ops independent,
                # so no dependency stalls and no scheduler gap-filling),
                # each seeded by a 2-line pair op, merged at the end.
                # Chain 0 owns lines 0..7 (acc slots 0/1), chain 1 lines
                # 8..15 (slots 2/3).
                acc = accpool.tile([128, 4 * cw], F32)

                def sl(i):
                    return acc[:, i * cw:(i + 1) * cw]

                h = hpool.tile([128, cw], F32)
                for c in (0, 1):
                    j0 = 8 * c
                    nc.vector._custom_dve(
                        fpair, out=sl(2 * c), in0=d, in1=cf[:, j0 + 1:j0 + 2],
                        s0=a[j0], s1=cf[:, j0:j0 + 1], imm2=a[j0 + 1],
                    )
                pos = [0, 2]
                for i in range(2, 8):
                    for c in (0, 1):
                        j = 8 * c + i
                        base = 2 * c
                        nxt = base + 1 - (pos[c] - base)
                        nc.vector._custom_dve(
                            fmin, out=sl(nxt), in0=d, in1=sl(pos[c]),
                            s0=a[j], s1=cf[:, j:j + 1],
                        )
                        pos[c] = nxt
                nc.vector.tensor_tensor(
                    h[:], sl(pos[0]), sl(pos[1]), mybir.AluOpType.min
                )

                # stage3: out[k] = relu(h - z_k). Engine split: tile0 halves
                # all on scalar (vector is chain-latency-bound there); the
                # last unit alternates k between engines so the final DMA
                # chunks fire ASAP; steady units give vector a small share.
                oa = opool.tile([128, FF * cw], F32)
                for k in range(FF):
                    o = oa[:, k * cw:(k + 1) * cw]
                    if ui == len(units) - 1:
                        on_vec = k % 2 == 1
                    elif ui <= 1:
                        on_vec = False
                    else:
                        on_vec = k >= FF - S3_VEC
                    if not on_vec:
                        nc.scalar.activation(
                            o, h[:], mybir.ActivationFunctionType.Relu,
                            bias=nz[:, k:k + 1], scale=1.0,
                        )
                    else:
                        nc.vector.tensor_scalar(
                            o, h[:], nz[:, k:k + 1], 0.0,
                            op0=mybir.AluOpType.add, op1=mybir.AluOpType.max,
                        )

                # stream output in ~1MB chunks (4 k's each) so stores overlap
                # stage3 and the final store doesn't serialize after compute
                ov = out_ext[rows, :].rearrange("p (k m) -> p k m", k=FF)
                for c in range(4):
                    nc.sync.dma_start(
                        ov[:, 4 * c:4 * (c + 1), c0:c1],
                        oa[:, 4 * c * cw:4 * (c + 1) * cw],
                    )
    nc.finalize()
    return nc


def _prep_inputs(u_coarse, topo):
    """Host-side: per-cell sorted-prefix coefficients + per-core shards."""
    u = np.ascontiguousarray(np.asarray(u_coarse, dtype=np.float32))
    tp = np.asarray(topo, dtype=np.float32)
    # fold fine topo into per-coarse-cell patches [NY, NX, FF]
    z = tp.reshape(NY, F, NX, F).transpose(0, 2, 1, 3).reshape(NY, NX, FF)
    zs = np.sort(z.astype(np.float64), axis=-1)
    pref = np.cumsum(zs, axis=-1)
    jj = np.arange(1, FF + 1, dtype=np.float64)
    coef = np.concatenate(
        [(pref / jj).astype(np.float32), (-z).astype(np.float32)], axis=-1
    )                                              # [NY, NX, 2*FF]

    in_maps = []
    for c in range(NCORES):
        ys = slice(c * YPC, (c + 1) * YPC)
        u_core = np.ascontiguousarray(
            u[:, ys, :, :].transpose(1, 2, 0, 3)
        ).reshape(CELLS, COMBOS)
        cf_core = np.ascontiguousarray(coef[ys]).reshape(CELLS, 2 * FF)
        in_maps.append({"u": u_core, "coef": cf_core})
    return in_maps


def _unshard(results):
    out_all = np.stack([r["out"] for r in results])          # [8, 512, 8192]
    arr = out_all.reshape(NCORES, YPC, NX, F, F, BS, NT)      # c,yl,x,fy,fx,b,t
    arr = arr.transpose(5, 0, 1, 3, 2, 4, 6)                  # b,c,yl,fy,x,fx,t
    return np.ascontiguousarray(arr).reshape(BS, NY * F, NX * F, NT)


def kernel(u_coarse, topo):
    if "nc" not in _CACHE:
        _CACHE["nc"] = _build_nc()
    nc = _CACHE["nc"]
    in_maps = _prep_inputs(u_coarse, topo)
    res = run_bass_kernel_spmd(nc, in_maps, core_ids=list(range(NCORES)))
    return _unshard(res.results)


if __name__ == "__main__":
    import reference

    inputs = reference.setup_inputs()
    out = kernel(**{k: np.asarray(v) for k, v in inputs.items()})
    print("out", out.shape, out.dtype)
